# revision 15
# baseline (speedup 1.0000x reference)
"""Chamfer (AutoEncoder) loss on 8 Trainium2 NeuronCores.

Problem: predictions [16, 2048, 3], targets [16, 2048, 3] (float32).
loss = sum_b [ sum_i min_j ||x_bi - y_bj||^2 + sum_j min_i ||x_bi - y_bj||^2 ]

Strategy (v2: single-P dual-fold)
--------------------------------
Data-parallel over the batch: 16 batches / 8 cores = 2 jobs per core.
Unlike v1 (which computed P and P^T separately, 2x matmul + paired DVE
reads + ACT copies), v2 computes each batch's distance matrix ONCE and
extracts BOTH reductions from a single DVE pass per strip:

  - a custom DVE op (hand-edited uop program) reads the PSUM strip
    (Src0) paired with the running column-accumulator (Src1, SBUF) and
    in ONE instruction emits:
        out      = max(Src0, Src1)   -> new column accumulator
        accum_out= max-fold(Src0)    -> per-row max of the strip
    (operands are negated, so max == min of distances).
  - after 16 strips the accumulator holds the column maxes of -P;
    16 PE transposes + one DVE tensor_reduce fold it across the
    partition axis into per-column results.

This halves PE work, frees the Scalar engine entirely, and runs the
DVE at its exact-arithmetic floor (every P element enters the DVE
exactly once; HW probes showed no 2x/4x DVE perf modes engage on this
toolchain, all engines reduce at 1 col/cycle, and only the DVE can
compute max at all).

fp32 matmul runs in LOW_HIGH mode (~8x slower than bf16), so operands
are split hi/lo in bf16 and K-stacked (hi*hi + hi*lo + lo*hi, K=15);
PE time scales with output columns, not K. PE row-group rotation
(partition offsets 0/32/64) keeps three sub-array pipelines running.
"""

import ml_dtypes
import numpy as np

import concourse.dve_ops as dve_ops
import concourse.mybir as mybir
import concourse.tile as tile
from concourse import bacc
from concourse.bass_utils import run_bass_kernel_spmd
from concourse.dve_ops import DveOp
from concourse.dve_spec import Spec, Src0, Src1, _has_src1, lower, maxx, minn
from concourse.dve_table_gen import dve_ver_for  # noqa: F401  (ver sanity)
from concourse.dve_uop import (
    DelayInp,
    DveOpSpec,
    OutPath,
    OutSel,
)


def _fold_free(a):
    return np.max(
        a.astype(np.float32), axis=tuple(range(1, a.ndim))
    ).reshape(a.shape[0], 1)


def _register_op(name, spec, edit=None):
    """Register a custom DVE op; optionally hand-edit the lowered uops.

    The edited program is injected into dve_ops._COMPILE_CACHE so both
    the per-NEFF table generator and the instruction emitter use it
    (DveOp.compile checks the cache before re-lowering; a cache miss
    would re-lower the spec and fail the pinned-sha check loudly).
    """
    for existing in dve_ops.OPS:
        if existing.name == name:
            return existing
    row = dve_ops._CUSTOM_DVE_ROW_BASE + len(dve_ops.OPS)
    shas = {}
    compiled = {}
    for ver in ("v3", "v4"):
        try:
            uops = lower(spec, ver=ver)
        except Exception:
            continue
        if edit is not None:
            uops = edit(uops, ver)
        s = DveOpSpec(name=name, opcode=row, uops=uops, rd1_en=_has_src1(spec))
        s.validate(ver)
        shas[ver] = s.sha(ver)
        compiled[ver] = s
    op = DveOp(name, spec, subdim=False, uops_sha=shas)
    dve_ops.OPS.append(op)
    dve_ops._SUB_OPCODE_FOR_NAME[op.name] = row
    dve_ops.CUSTOM_DVE_SPECS[op.name] = op.spec
    for ver, s in compiled.items():
        dve_ops._COMPILE_CACHE[(name, ver)] = s
    assert max(dve_ops._SUB_OPCODE_FOR_NAME.values()) < 0x20
    return op


def _register_maxpair_fold():
    """out = max(Src0, Src1); accum_out = max-fold(Src0).

    The Spec DSL can only fold the body root, so the body is
    min(Src0, max(Src0, Src1)) == Src0 — lower() then naturally builds:
      dp[0]: MAX(Src0, Src1)      (the pair max)
      dp[1]: MIN(Src0, PREV)      (== Src0, the fold input)
      dp[2]: accumulator MAX(CURR, PREV)  -> accum_out = fold(Src0)
      out   = DELAY_0 (captured root == Src0)
    The hand edit reroutes `out` to the dp[0] pair max: capture it into
    free delay lane 3 at dp[1] and select DELAY_3 as the write source.
    """

    def edit(uops, ver):
        assert len(uops) == 2, f"expected seed+steady, got {len(uops)}"
        seed, steady = uops
        assert steady.require_inp0 == 1, "uop order changed"
        for u in uops:
            for dp in u.datapath_config:
                dp.delay[3] = DelayInp.PREV_DELAY
                dp.delay_enable[3] = 1
        # capture dp[0]'s ALU out (the pair max) into lane 3 at stage 1
        steady.datapath_config[1].delay[3] = DelayInp.PREV_ALU_OUT
        steady.out[OutPath.WR0_LO] = OutSel.DELAY_3
        return uops

    spec = Spec(
        body=minn(Src0, maxx(Src0, Src1)),
        accum=maxx,
        reference=lambda in0, in1, s0, s1, imm2: (
            np.maximum(in0.astype(np.float32), in1.astype(np.float32)),
            _fold_free(in0),
        ),
    )
    return _register_op("MAXPAIR_FOLD0_ANT", spec, edit)


def _register_copy_fold():
    """out = Src0 (accumulator init); accum_out = max-fold(Src0)."""
    spec = Spec(
        body=Src0,
        accum=maxx,
        reference=lambda in0, in1, s0, s1, imm2: (
            in0.astype(np.float32),
            _fold_free(in0),
        ),
    )
    return _register_op("COPY_FOLD0_ANT", spec)


MAXPAIR_FOLD = _register_maxpair_fold()
COPY_FOLD = _register_copy_fold()

B, N, M, D = 16, 2048, 2048, 3
N_CORES = 8
JOBS = B // N_CORES  # batches per core (2); one job per batch
ROW_TILES = N // 128  # 16
COL_CHUNK = 512
KCAT = 15  # [hi; hi; lo] x [hi; lo; hi]

_F32 = mybir.dt.float32
_BF16 = mybir.dt.bfloat16
_NP_BF16 = ml_dtypes.bfloat16

_cached_nc = None


def _build_nc():
    nc = bacc.Bacc("TRN2", target_bir_lowering=False, debug=False)
    # lhs+rhs packed per replica so one DMA per replica loads both
    ops = nc.dram_tensor("ops", [JOBS, 3, KCAT, 2 * N], _BF16, kind="ExternalInput")
    # 2 cols per strip (lo/hi half folds; host maxes the pair)
    rowm = nc.dram_tensor(
        "rowm", [JOBS, 128, 2 * ROW_TILES], _F32, kind="ExternalOutput"
    )
    # final column accumulators, folded over rows on the host (job 0's DMA
    # hides under job 1's compute; job 1's halves ship as they finalize)
    accs = nc.dram_tensor("accs", [JOBS, 128, M], _F32, kind="ExternalOutput")

    with tile.TileContext(nc) as tc:
        with (
            tc.tile_pool(name="inp", bufs=3) as inp_pool,
            tc.tile_pool(name="psum", bufs=2, space="PSUM") as psum_pool,
            tc.tile_pool(name="acc", bufs=2) as acc_pool,
            tc.tile_pool(name="res", bufs=2) as res_pool,
        ):
            for j in range(JOBS):
                ops_sb = inp_pool.tile([128, 2 * N], _BF16, tag="ops")
                # Operand replicas at partition offsets 0/32/64 for PE
                # row-group rotation: one packed lhs+rhs DMA per replica,
                # one initiator engine each (ramp).
                engines = (nc.sync, nc.scalar, nc.gpsimd) if j == 0 else (nc.sync,) * 3
                for a, g in enumerate((0, 32, 64)):
                    engines[a].dma_start(ops_sb[g : g + KCAT, :], ops[j, a])

                rowm_sb = res_pool.tile([128, 2 * ROW_TILES], _F32, tag="rowm")
                acc_a = acc_pool.tile([128, M], _F32, tag="acc_a")
                acc_b = acc_pool.tile([128, M], _F32, tag="acc_b")

                H = M // 2  # DVE/PSUM access patterns must stay <= 2 banks
                for i in range(ROW_TILES):
                    lo_ps = psum_pool.tile([128, H], _F32, tag="lo")
                    hi_ps = psum_pool.tile([128, H], _F32, tag="hi")
                    li = slice(i * 128, (i + 1) * 128)
                    for c in range(4):
                        # Strip 0 of job 0 alternates groups 0/32 only so its
                        # matmuls gate on two DMA queues, not three (ramp).
                        if j == 0 and i == 0:
                            g = (c % 2) * 32
                        else:
                            g = ((i * 4 + c) % 3) * 32
                        cs = slice(c * COL_CHUNK, (c + 1) * COL_CHUNK)
                        dst = lo_ps if c < 2 else hi_ps
                        ds = slice((c % 2) * COL_CHUNK, (c % 2 + 1) * COL_CHUNK)
                        nc.tensor.matmul(
                            dst[:, ds],
                            ops_sb[g : g + KCAT, li],
                            ops_sb[g : g + KCAT, N + cs.start : N + cs.stop],
                            start=True,
                            stop=True,
                        )
                    cur, prv = (acc_a, acc_b) if i % 2 == 0 else (acc_b, acc_a)
                    for h, ps in ((0, lo_ps), (1, hi_ps)):
                        hs = slice(h * H, (h + 1) * H)
                        rs = slice(2 * i + h, 2 * i + h + 1)
                        if i == 0:
                            nc.vector._custom_dve(
                                COPY_FOLD,
                                out=cur[:, hs],
                                in0=ps[:],
                                accum_out=rowm_sb[:, rs],
                            )
                        else:
                            nc.vector._custom_dve(
                                MAXPAIR_FOLD,
                                out=cur[:, hs],
                                in0=ps[:],
                                in1=prv[:, hs],
                                accum_out=rowm_sb[:, rs],
                            )
                final_acc = acc_a if (ROW_TILES - 1) % 2 == 0 else acc_b

                # Host-side column fold: DMA each final accumulator half as
                # soon as its last strip instruction retires (lo finalizes
                # one instruction before hi).
                nc.scalar.dma_start(accs[j][:, 0:H], final_acc[:, 0:H])
                nc.sync.dma_start(accs[j][:, H:M], final_acc[:, H:M])
                nc.gpsimd.dma_start(rowm[j], rowm_sb[:])
    nc.compile()
    return nc


def _get_nc():
    global _cached_nc
    if _cached_nc is None:
        _cached_nc = _build_nc()
    return _cached_nc


def _augment(a, b):
    """a: [n, 3], b: [m, 3] -> (lhsT [5, n], rhs [5, m]) float32.

    lhsT is negated so the device matmul yields -P.
    """
    n = a.shape[0]
    m = b.shape[0]
    lhsT = np.empty((5, n), dtype=np.float32)
    lhsT[0:3] = -a.T
    lhsT[3] = -(a * a).sum(axis=1)
    lhsT[4] = -1.0
    rhs = np.empty((5, m), dtype=np.float32)
    rhs[0:3] = -2.0 * b.T
    rhs[3] = 1.0
    rhs[4] = (b * b).sum(axis=1)
    return lhsT, rhs


def _split_cat(lhs, rhs):
    """fp32 [J, 5, n] operands -> K-stacked bf16 [J, 3, 15, n] hi/lo forms."""
    lh = lhs.astype(_NP_BF16)
    ll = (lhs - lh.astype(np.float32)).astype(_NP_BF16)
    rh = rhs.astype(_NP_BF16)
    rl = (rhs - rh.astype(np.float32)).astype(_NP_BF16)
    lcat = np.concatenate([lh, lh, ll], axis=1)
    rcat = np.concatenate([rh, rl, rh], axis=1)
    packed = np.concatenate([lcat, rcat], axis=2)  # [J, KCAT, 2*N]
    return np.ascontiguousarray(np.repeat(packed[:, None, :, :], 3, axis=1))


def _in_maps(predictions, targets):
    in_maps = []
    for core in range(N_CORES):
        lhs = np.empty((JOBS, 5, N), dtype=np.float32)
        rhs = np.empty((JOBS, 5, M), dtype=np.float32)
        for j in range(JOBS):
            b = core * JOBS + j
            lhs[j], rhs[j] = _augment(predictions[b], targets[b])
        in_maps.append({"ops": _split_cat(lhs, rhs)})
    return in_maps


def _host_reduce(results):
    """Sum per-core rowm/colm outputs into the final scalar loss.

    rowm holds separate lo/hi half folds per strip (cols 2i / 2i+1);
    the row max is the max of the pair.
    """
    total = 0.0
    for core in range(N_CORES):
        rowm = results[core]["rowm"].astype(np.float64)
        pairs = rowm.reshape(JOBS, 128, ROW_TILES, 2)
        total -= pairs.max(axis=-1).sum()
        total -= results[core]["accs"].astype(np.float64).max(axis=1).sum()
    return np.float32(total)


def kernel(predictions, targets):
    predictions = np.asarray(predictions, dtype=np.float32)
    targets = np.asarray(targets, dtype=np.float32)

    nc = _get_nc()
    res = run_bass_kernel_spmd(
        nc, _in_maps(predictions, targets), core_ids=list(range(N_CORES))
    )
    return _host_reduce(res.results)


# revision 16
# speedup vs baseline: 1.1412x; 1.1412x over previous
"""Chamfer (AutoEncoder) loss on 8 Trainium2 NeuronCores.

Problem: predictions [16, 2048, 3], targets [16, 2048, 3] (float32).
loss = sum_b [ sum_i min_j ||x_bi - y_bj||^2 + sum_j min_i ||x_bi - y_bj||^2 ]

Strategy (v2: single-P dual-fold)
--------------------------------
Data-parallel over the batch: 16 batches / 8 cores = 2 jobs per core.
Unlike v1 (which computed P and P^T separately, 2x matmul + paired DVE
reads + ACT copies), v2 computes each batch's distance matrix ONCE and
extracts BOTH reductions from a single DVE pass per strip:

  - a custom DVE op (hand-edited uop program) reads the PSUM strip
    (Src0) paired with the running column-accumulator (Src1, SBUF) and
    in ONE instruction emits:
        out      = max(Src0, Src1)   -> new column accumulator
        accum_out= max-fold(Src0)    -> per-row max of the strip
    (operands are negated, so max == min of distances).
  - after 16 strips the accumulator holds the column maxes of -P;
    16 PE transposes + one DVE tensor_reduce fold it across the
    partition axis into per-column results.

This halves PE work, frees the Scalar engine entirely, and runs the
DVE at its exact-arithmetic floor (every P element enters the DVE
exactly once; HW probes showed no 2x/4x DVE perf modes engage on this
toolchain, all engines reduce at 1 col/cycle, and only the DVE can
compute max at all).

fp32 matmul runs in LOW_HIGH mode (~8x slower than bf16), so operands
are split hi/lo in bf16 and K-stacked (hi*hi + hi*lo + lo*hi, K=15);
PE time scales with output columns, not K. PE row-group rotation
(partition offsets 0/32/64) keeps three sub-array pipelines running.
"""

import ml_dtypes
import numpy as np

import concourse.dve_ops as dve_ops
import concourse.mybir as mybir
import concourse.tile as tile
from concourse import bacc
from concourse.bass_utils import run_bass_kernel_spmd
from concourse.dve_ops import DveOp
from concourse.dve_spec import Spec, Src0, Src1, _has_src1, lower, maxx, minn
from concourse.dve_table_gen import dve_ver_for  # noqa: F401  (ver sanity)
from concourse.dve_uop import (
    DelayInp,
    DveOpSpec,
    OutPath,
    OutSel,
)


def _fold_free(a):
    return np.max(
        a.astype(np.float32), axis=tuple(range(1, a.ndim))
    ).reshape(a.shape[0], 1)


def _register_op(name, spec, edit=None):
    """Register a custom DVE op; optionally hand-edit the lowered uops.

    The edited program is injected into dve_ops._COMPILE_CACHE so both
    the per-NEFF table generator and the instruction emitter use it
    (DveOp.compile checks the cache before re-lowering; a cache miss
    would re-lower the spec and fail the pinned-sha check loudly).
    """
    for existing in dve_ops.OPS:
        if existing.name == name:
            return existing
    row = dve_ops._CUSTOM_DVE_ROW_BASE + len(dve_ops.OPS)
    shas = {}
    compiled = {}
    for ver in ("v3", "v4"):
        try:
            uops = lower(spec, ver=ver)
        except Exception:
            continue
        if edit is not None:
            uops = edit(uops, ver)
        s = DveOpSpec(name=name, opcode=row, uops=uops, rd1_en=_has_src1(spec))
        s.validate(ver)
        shas[ver] = s.sha(ver)
        compiled[ver] = s
    op = DveOp(name, spec, subdim=False, uops_sha=shas)
    dve_ops.OPS.append(op)
    dve_ops._SUB_OPCODE_FOR_NAME[op.name] = row
    dve_ops.CUSTOM_DVE_SPECS[op.name] = op.spec
    for ver, s in compiled.items():
        dve_ops._COMPILE_CACHE[(name, ver)] = s
    assert max(dve_ops._SUB_OPCODE_FOR_NAME.values()) < 0x20
    return op


def _register_maxpair_fold():
    """out = max(Src0, Src1); accum_out = max-fold(Src0).

    The Spec DSL can only fold the body root, so the body is
    min(Src0, max(Src0, Src1)) == Src0 — lower() then naturally builds:
      dp[0]: MAX(Src0, Src1)      (the pair max)
      dp[1]: MIN(Src0, PREV)      (== Src0, the fold input)
      dp[2]: accumulator MAX(CURR, PREV)  -> accum_out = fold(Src0)
      out   = DELAY_0 (captured root == Src0)
    The hand edit reroutes `out` to the dp[0] pair max: capture it into
    free delay lane 3 at dp[1] and select DELAY_3 as the write source.
    """

    def edit(uops, ver):
        assert len(uops) == 2, f"expected seed+steady, got {len(uops)}"
        seed, steady = uops
        assert steady.require_inp0 == 1, "uop order changed"
        for u in uops:
            for dp in u.datapath_config:
                dp.delay[3] = DelayInp.PREV_DELAY
                dp.delay_enable[3] = 1
        # capture dp[0]'s ALU out (the pair max) into lane 3 at stage 1
        steady.datapath_config[1].delay[3] = DelayInp.PREV_ALU_OUT
        steady.out[OutPath.WR0_LO] = OutSel.DELAY_3
        return uops

    spec = Spec(
        body=minn(Src0, maxx(Src0, Src1)),
        accum=maxx,
        reference=lambda in0, in1, s0, s1, imm2: (
            np.maximum(in0.astype(np.float32), in1.astype(np.float32)),
            _fold_free(in0),
        ),
    )
    return _register_op("MAXPAIR_FOLD0_ANT", spec, edit)


def _register_copy_fold():
    """out = Src0 (accumulator init); accum_out = max-fold(Src0)."""
    spec = Spec(
        body=Src0,
        accum=maxx,
        reference=lambda in0, in1, s0, s1, imm2: (
            in0.astype(np.float32),
            _fold_free(in0),
        ),
    )
    return _register_op("COPY_FOLD0_ANT", spec)


MAXPAIR_FOLD = _register_maxpair_fold()
COPY_FOLD = _register_copy_fold()

B, N, M, D = 16, 2048, 2048, 3
N_CORES = 8
JOBS = B // N_CORES  # batches per core (2); one job per batch
ROW_TILES = N // 128  # 16
COL_CHUNK = 512
KCAT = 15  # [hi; hi; lo] x [hi; lo; hi]

_F32 = mybir.dt.float32
_BF16 = mybir.dt.bfloat16
_NP_BF16 = ml_dtypes.bfloat16

_cached_nc = None


def _build_nc():
    nc = bacc.Bacc("TRN2", target_bir_lowering=False, debug=False)
    # lhs+rhs packed per replica so one DMA per replica loads both
    ops = nc.dram_tensor("ops", [JOBS, 3, KCAT, 2 * N], _BF16, kind="ExternalInput")
    # 2 cols per strip (lo/hi half folds; host maxes the pair)
    rowm = nc.dram_tensor(
        "rowm", [JOBS, 128, 2 * ROW_TILES], _F32, kind="ExternalOutput"
    )
    colm = nc.dram_tensor("colm", [128, ROW_TILES], _F32, kind="ExternalOutput")
    # job 0's final column accumulator, reduced on host (the DMA hides
    # under job 1's compute; only job 1 needs the on-device tail)
    acc0 = nc.dram_tensor("acc0", [128, M], _F32, kind="ExternalOutput")
    ident = nc.dram_tensor("ident", [128, 128], _F32, kind="ExternalInput")

    with tile.TileContext(nc) as tc:
        with (
            tc.tile_pool(name="inp", bufs=3) as inp_pool,
            tc.tile_pool(name="psum", bufs=2, space="PSUM") as psum_pool,
            tc.tile_pool(name="acc", bufs=2) as acc_pool,
            tc.tile_pool(name="res", bufs=2) as res_pool,
            tc.tile_pool(name="one", bufs=1) as one_pool,
        ):
            ident_sb = one_pool.tile([128, 128], _F32, tag="ident")
            for j in range(JOBS):
                ops_sb = inp_pool.tile([128, 2 * N], _BF16, tag="ops")
                # Operand replicas at partition offsets 0/32/64 for PE
                # row-group rotation: one packed lhs+rhs DMA per replica,
                # one initiator engine each (ramp).
                engines = (nc.sync, nc.scalar, nc.gpsimd) if j == 0 else (nc.sync,) * 3
                for a, g in enumerate((0, 32, 64)):
                    engines[a].dma_start(ops_sb[g : g + KCAT, :], ops[j, a])
                if j == 0:
                    nc.gpsimd.dma_start(ident_sb[:], ident[:, :])

                rowm_sb = res_pool.tile([128, 2 * ROW_TILES], _F32, tag="rowm")
                colm_sb = res_pool.tile([128, ROW_TILES], _F32, tag="colm")
                acc_a = acc_pool.tile([128, M], _F32, tag="acc_a")
                acc_b = acc_pool.tile([128, M], _F32, tag="acc_b")

                H = M // 2  # DVE/PSUM access patterns must stay <= 2 banks
                for i in range(ROW_TILES):
                    lo_ps = psum_pool.tile([128, H], _F32, tag="lo")
                    hi_ps = psum_pool.tile([128, H], _F32, tag="hi")
                    li = slice(i * 128, (i + 1) * 128)
                    for c in range(4):
                        # Strip 0 of job 0 alternates groups 0/32 only so its
                        # matmuls gate on two DMA queues, not three (ramp).
                        if j == 0 and i == 0:
                            g = (c % 2) * 32
                        else:
                            g = ((i * 4 + c) % 3) * 32
                        cs = slice(c * COL_CHUNK, (c + 1) * COL_CHUNK)
                        dst = lo_ps if c < 2 else hi_ps
                        ds = slice((c % 2) * COL_CHUNK, (c % 2 + 1) * COL_CHUNK)
                        nc.tensor.matmul(
                            dst[:, ds],
                            ops_sb[g : g + KCAT, li],
                            ops_sb[g : g + KCAT, N + cs.start : N + cs.stop],
                            start=True,
                            stop=True,
                        )
                    cur, prv = (acc_a, acc_b) if i % 2 == 0 else (acc_b, acc_a)
                    for h, ps in ((0, lo_ps), (1, hi_ps)):
                        hs = slice(h * H, (h + 1) * H)
                        rs = slice(2 * i + h, 2 * i + h + 1)
                        if i == 0:
                            nc.vector._custom_dve(
                                COPY_FOLD,
                                out=cur[:, hs],
                                in0=ps[:],
                                accum_out=rowm_sb[:, rs],
                            )
                        else:
                            nc.vector._custom_dve(
                                MAXPAIR_FOLD,
                                out=cur[:, hs],
                                in0=ps[:],
                                in1=prv[:, hs],
                                accum_out=rowm_sb[:, rs],
                            )
                final_acc = acc_a if (ROW_TILES - 1) % 2 == 0 else acc_b

                if j == 0:
                    # Host-side column fold for job 0: DMA the accumulator
                    # out across three idle queues; hides under job 1.
                    nc.sync.dma_start(acc0[:, 0:682], final_acc[:, 0:682])
                    nc.scalar.dma_start(acc0[:, 682:1364], final_acc[:, 682:1364])
                    nc.gpsimd.dma_start(acc0[:, 1364:M], final_acc[:, 1364:M])
                else:
                    # Column maxes: 16 PE transposes of the accumulator into
                    # two PSUM tiles [128, 8, 128], each folded over the
                    # innermost (original partition) axis by a tensor_reduce.
                    for h in range(2):
                        tp = psum_pool.tile(
                            [128, 8, 128], _F32, tag="lo" if h == 0 else "hi"
                        )
                        for t in range(8):
                            k = h * 8 + t
                            nc.tensor.transpose(
                                tp[:, t, :],
                                final_acc[:, k * 128 : (k + 1) * 128],
                                ident_sb[:],
                            )
                        nc.vector.tensor_reduce(
                            colm_sb[:, h * 8 : (h + 1) * 8],
                            tp[:],
                            mybir.AxisListType.X,
                            mybir.AluOpType.max,
                        )
                    nc.sync.dma_start(colm[:, :], colm_sb[:])
                nc.sync.dma_start(rowm[j], rowm_sb[:])
    nc.compile()
    return nc


def _get_nc():
    global _cached_nc
    if _cached_nc is None:
        _cached_nc = _build_nc()
    return _cached_nc


def _augment(a, b):
    """a: [n, 3], b: [m, 3] -> (lhsT [5, n], rhs [5, m]) float32.

    lhsT is negated so the device matmul yields -P.
    """
    n = a.shape[0]
    m = b.shape[0]
    lhsT = np.empty((5, n), dtype=np.float32)
    lhsT[0:3] = -a.T
    lhsT[3] = -(a * a).sum(axis=1)
    lhsT[4] = -1.0
    rhs = np.empty((5, m), dtype=np.float32)
    rhs[0:3] = -2.0 * b.T
    rhs[3] = 1.0
    rhs[4] = (b * b).sum(axis=1)
    return lhsT, rhs


def _split_cat(lhs, rhs):
    """fp32 [J, 5, n] operands -> K-stacked bf16 [J, 3, 15, n] hi/lo forms."""
    lh = lhs.astype(_NP_BF16)
    ll = (lhs - lh.astype(np.float32)).astype(_NP_BF16)
    rh = rhs.astype(_NP_BF16)
    rl = (rhs - rh.astype(np.float32)).astype(_NP_BF16)
    lcat = np.concatenate([lh, lh, ll], axis=1)
    rcat = np.concatenate([rh, rl, rh], axis=1)
    packed = np.concatenate([lcat, rcat], axis=2)  # [J, KCAT, 2*N]
    return np.ascontiguousarray(np.repeat(packed[:, None, :, :], 3, axis=1))


_IDENT = np.eye(128, dtype=np.float32)


def _in_maps(predictions, targets):
    in_maps = []
    for core in range(N_CORES):
        lhs = np.empty((JOBS, 5, N), dtype=np.float32)
        rhs = np.empty((JOBS, 5, M), dtype=np.float32)
        for j in range(JOBS):
            b = core * JOBS + j
            lhs[j], rhs[j] = _augment(predictions[b], targets[b])
        in_maps.append({"ops": _split_cat(lhs, rhs), "ident": _IDENT})
    return in_maps


def _host_reduce(results):
    """Sum per-core rowm/colm outputs into the final scalar loss.

    rowm holds separate lo/hi half folds per strip (cols 2i / 2i+1);
    the row max is the max of the pair.
    """
    total = 0.0
    for core in range(N_CORES):
        rowm = results[core]["rowm"].astype(np.float64)
        pairs = rowm.reshape(JOBS, 128, ROW_TILES, 2)
        total -= pairs.max(axis=-1).sum()
        total -= results[core]["colm"].astype(np.float64).sum()
        total -= results[core]["acc0"].astype(np.float64).max(axis=0).sum()
    return np.float32(total)


def kernel(predictions, targets):
    predictions = np.asarray(predictions, dtype=np.float32)
    targets = np.asarray(targets, dtype=np.float32)

    nc = _get_nc()
    res = run_bass_kernel_spmd(
        nc, _in_maps(predictions, targets), core_ids=list(range(N_CORES))
    )
    return _host_reduce(res.results)


# revision 17
# speedup vs baseline: 1.1542x; 1.0114x over previous
"""Chamfer (AutoEncoder) loss on 8 Trainium2 NeuronCores.

Problem: predictions [16, 2048, 3], targets [16, 2048, 3] (float32).
loss = sum_b [ sum_i min_j ||x_bi - y_bj||^2 + sum_j min_i ||x_bi - y_bj||^2 ]

Strategy
--------
Data-parallel over the batch: 16 batches / 8 cores = 2 per core. Each
(batch, direction) pair is one of 4 identical "jobs" per core.

The pairwise squared-distance matrix is produced directly by K-stacked
matmuls via the augmentation trick (negated so the device computes -P
and all minima become maxima):
    a' = -[a0, a1, a2, |a|^2, 1]           (5 x n)
    b' =  [-2*b0, -2*b1, -2*b2, 1, |b|^2]  (5 x m)
    a'.T @ b' = -P,  P[i, j] = |a_i|^2 + |b_j|^2 - 2 a_i.b_j

fp32 matmul on TRN2 runs in LOW_HIGH mode (~8x slower than bf16), so
operands are split hi/lo in bf16 and the three product terms
(hi*hi + hi*lo + lo*hi) are stacked along the contraction dim:
    lhsT = [a'_hi; a'_hi; a'_lo]  (15 x n, bf16)
    rhs  = [b'_hi; b'_lo; b'_hi]  (15 x m, bf16)
One K=15 bf16 matmul per output tile then yields -P at ~fp32 precision
(PE time scales with output columns, not K; PSUM accumulates in fp32;
the dropped lo*lo term is O(2^-17) relative).

PE row-group rotation: the K=15 weights occupy one 32-row group of the
128x128 array. Operands are replicated at partition offsets 0/32/64 and
consecutive output tiles rotate across those three row groups, so three
sub-array pipelines run concurrently (~3x matmul issue rate) and each
LDWEIGHTS overlaps other groups' in-flight matmuls.

Each job: 16 row-strips of [128, 2048] built by 4 matmuls (one PSUM bank
each). The strip is drained by two engines in parallel: ACT copies the
upper half to SBUF while a single custom-DVE MAX2_REDUCE instruction
consumes the PSUM lower half paired with that SBUF copy (2 elements per
DVE cycle), max-folding into one column of a [128, 16] accumulator.
Per-core output is [4, 128, 16] of -min; the host sums and negates
(the final scalar all-reduce) and returns the float32 scalar.
"""

import ml_dtypes
import numpy as np

import concourse.dve_ops as dve_ops
import concourse.mybir as mybir
import concourse.tile as tile
from concourse import bacc
from concourse.bass_utils import run_bass_kernel_spmd
from concourse.dve_ops import DveOp
from concourse.dve_spec import Spec, Src0, Src1, _has_src1, lower, maxx
from concourse.dve_table_gen import dve_ver_for  # noqa: F401  (ver sanity)
from concourse.dve_uop import DveOpSpec


def _register_max2() -> DveOp:
    """Custom DVE op: body = max(Src0, Src1), accum_out = max fold.

    Consumes two tensor streams per cycle (one may be PSUM); with negated
    inputs this is a paired min-reduction. Registered into the live
    concourse.dve_ops tables (the per-NEFF DVE table generator resolves
    ops by name from dve_ops.OPS).
    """
    for existing in dve_ops.OPS:
        if existing.name == "MAX2_REDUCE_ANT":
            return existing
    spec = Spec(
        body=maxx(Src0, Src1),
        accum=maxx,
        reference=lambda in0, in1, s0, s1, imm2: (
            np.maximum(in0.astype(np.float32), in1.astype(np.float32)),
            np.maximum.reduce(
                np.maximum(in0.astype(np.float32), in1.astype(np.float32)),
                axis=tuple(range(1, in0.ndim)),
            ).reshape(in0.shape[0], 1),
        ),
    )
    name = "MAX2_REDUCE_ANT"
    row = dve_ops._CUSTOM_DVE_ROW_BASE + len(dve_ops.OPS)
    shas = {}
    for ver in ("v3", "v4"):
        try:
            uops = lower(spec, ver=ver)
        except Exception:
            continue
        shas[ver] = DveOpSpec(
            name=name, opcode=row, uops=uops, rd1_en=_has_src1(spec)
        ).sha(ver)
    op = DveOp(name, spec, subdim=False, uops_sha=shas)
    dve_ops.OPS.append(op)
    dve_ops._SUB_OPCODE_FOR_NAME[op.name] = row
    dve_ops.CUSTOM_DVE_SPECS[op.name] = op.spec
    assert max(dve_ops._SUB_OPCODE_FOR_NAME.values()) < 0x20
    return op


MAX2_REDUCE = _register_max2()

B, N, M, D = 16, 2048, 2048, 3
N_CORES = 8
BPC = B // N_CORES  # batches per core
JOBS = 2 * BPC  # (batch, direction) pairs per core
ROW_TILES = N // 128  # 16
COL_CHUNK = 512
COL_CHUNKS = M // COL_CHUNK  # 4
KCAT = 15  # [hi; hi; lo] x [hi; lo; hi]

_F32 = mybir.dt.float32
_BF16 = mybir.dt.bfloat16
_NP_BF16 = ml_dtypes.bfloat16

_cached_nc = None


def _build_nc():
    nc = bacc.Bacc("TRN2", target_bir_lowering=False, debug=False)
    lhs = nc.dram_tensor("lhs", [JOBS, 3, KCAT, N], _BF16, kind="ExternalInput")
    rhs = nc.dram_tensor("rhs", [JOBS, 3, KCAT, M], _BF16, kind="ExternalInput")
    out = nc.dram_tensor("maxs", [JOBS, 128, ROW_TILES], _F32, kind="ExternalOutput")

    with tile.TileContext(nc) as tc:
        with (
            tc.tile_pool(name="inp", bufs=3) as inp_pool,
            tc.tile_pool(name="psum", bufs=2, space="PSUM") as psum_pool,
            tc.tile_pool(name="acc", bufs=4) as acc_pool,
            tc.tile_pool(name="upper", bufs=8) as upper_pool,
        ):
            for j in range(JOBS):
                # Operands replicated at partition offsets 0/32/64 for PE
                # row-group rotation.
                lhs_sb = inp_pool.tile([128, N], _BF16, tag="lhs")
                rhs_sb = inp_pool.tile([128, M], _BF16, tag="rhs")
                # Replicas at partition offsets 0/32/64 for PE row-group
                # rotation. Job 0\'s loads gate the pipeline ramp, so spread
                # them across three engines\' DMA queues; later jobs\' loads
                # hide under compute on the sync queue.
                engines = (nc.sync, nc.scalar, nc.gpsimd) if j == 0 else (nc.sync,) * 3
                for a, g in enumerate((0, 32, 64)):
                    engines[a].dma_start(lhs_sb[g : g + KCAT, :], lhs[j, a])
                    engines[a].dma_start(rhs_sb[g : g + KCAT, :], rhs[j, a])
                maxs_sb = acc_pool.tile([128, ROW_TILES], _F32, tag="maxs")
                for i in range(ROW_TILES):
                    dummy = upper_pool.tile([128, 1], _F32, tag="dummy")
                    # Separate PSUM tiles for the ACT-drained upper half and
                    # the DVE-drained lower half so each recycles as soon as
                    # its own reader finishes (deeper pipeline than one
                    # monolithic 4-bank strip).
                    hi_ps = psum_pool.tile([128, M // 2], _F32, tag="hi")
                    lo_ps = psum_pool.tile([128, M // 2], _F32, tag="lo")
                    li = slice(i * 128, (i + 1) * 128)
                    # Upper-half banks first so the ACT copy can start while
                    # PE fills the lower half; chunks rotate row groups.
                    for k, (dst, half) in enumerate(
                        ((hi_ps, 0), (hi_ps, 1), (lo_ps, 0), (lo_ps, 1))
                    ):
                        g = ((i * 4 + k) % 3) * 32
                        c = 2 + k if k < 2 else k - 2
                        cs = slice(c * COL_CHUNK, (c + 1) * COL_CHUNK)
                        nc.tensor.matmul(
                            dst[:, half * COL_CHUNK : (half + 1) * COL_CHUNK],
                            lhs_sb[g : g + KCAT, li],
                            rhs_sb[g : g + KCAT, cs],
                            start=True,
                            stop=True,
                        )
                        if k == 1:
                            upper = upper_pool.tile([128, M // 2], _F32, tag="upper")
                            nc.scalar.copy(upper[:], hi_ps[:])
                    # One DVE instruction drains the PSUM lower half paired
                    # with the SBUF upper copy: accum = max over the strip.
                    nc.vector._custom_dve(
                        MAX2_REDUCE,
                        out=dummy.broadcast_to((128, M // 2)),
                        in0=lo_ps[:],
                        in1=upper[:],
                        accum_out=maxs_sb[:, i : i + 1],
                    )
                nc.sync.dma_start(out[j], maxs_sb[:])
    nc.compile()
    return nc


def _get_nc():
    global _cached_nc
    if _cached_nc is None:
        _cached_nc = _build_nc()
    return _cached_nc


def _augment(a, b):
    """a: [n, 3], b: [m, 3] -> (lhsT [5, n], rhs [5, m]) float32.

    lhsT is negated so the device matmul yields -P.
    """
    n = a.shape[0]
    m = b.shape[0]
    lhsT = np.empty((5, n), dtype=np.float32)
    lhsT[0:3] = -a.T
    lhsT[3] = -(a * a).sum(axis=1)
    lhsT[4] = -1.0
    rhs = np.empty((5, m), dtype=np.float32)
    rhs[0:3] = -2.0 * b.T
    rhs[3] = 1.0
    rhs[4] = (b * b).sum(axis=1)
    return lhsT, rhs


def _split_cat(lhs, rhs):
    """fp32 [J, 5, n] operands -> K-stacked bf16 [J, 15, n] hi/lo forms."""
    lh = lhs.astype(_NP_BF16)
    ll = (lhs - lh.astype(np.float32)).astype(_NP_BF16)
    rh = rhs.astype(_NP_BF16)
    rl = (rhs - rh.astype(np.float32)).astype(_NP_BF16)
    lcat = np.concatenate([lh, lh, ll], axis=1)
    rcat = np.concatenate([rh, rl, rh], axis=1)
    # Replicate for the three PE row groups (partition offsets 0/32/64).
    lrep = np.repeat(lcat[:, None, :, :], 3, axis=1)
    rrep = np.repeat(rcat[:, None, :, :], 3, axis=1)
    return np.ascontiguousarray(lrep), np.ascontiguousarray(rrep)


def _in_maps(predictions, targets):
    in_maps = []
    for core in range(N_CORES):
        lhs = np.empty((JOBS, 5, N), dtype=np.float32)
        rhs = np.empty((JOBS, 5, M), dtype=np.float32)
        for bi in range(BPC):
            b = core * BPC + bi
            # direction 0: rows = predictions, min over targets
            lhs[2 * bi], rhs[2 * bi] = _augment(predictions[b], targets[b])
            # direction 1: rows = targets, min over predictions
            lhs[2 * bi + 1], rhs[2 * bi + 1] = _augment(targets[b], predictions[b])
        lcat, rcat = _split_cat(lhs, rhs)
        in_maps.append({"lhs": lcat, "rhs": rcat})
    return in_maps


def _host_reduce(results):
    total = 0.0
    for core in range(N_CORES):
        total -= results[core]["maxs"].astype(np.float64).sum()
    return np.float32(total)


def kernel(predictions, targets):
    predictions = np.asarray(predictions, dtype=np.float32)
    targets = np.asarray(targets, dtype=np.float32)

    nc = _get_nc()
    res = run_bass_kernel_spmd(
        nc, _in_maps(predictions, targets), core_ids=list(range(N_CORES))
    )
    return _host_reduce(res.results)


# revision 18
# speedup vs baseline: 1.1637x; 1.0082x over previous
"""Chamfer (AutoEncoder) loss on 8 Trainium2 NeuronCores.

Problem: predictions [16, 2048, 3], targets [16, 2048, 3] (float32).
loss = sum_b [ sum_i min_j ||x_bi - y_bj||^2 + sum_j min_i ||x_bi - y_bj||^2 ]

Strategy (v3: hybrid dual-fold / paired-fold)
---------------------------------------------
Data-parallel over the batch: 2 batches per core. The DVE is the only
engine that can compute max (HW probes: ACT accumulates sums only,
GPSIMD rejects tensor-tensor ops, no DVE 2x perf modes engage), so
every distance-matrix element must stream through it; the whole design
minimizes DVE instruction count and kernel tail.

- Batch 0 (job 0, "single-P dual-fold"): each [128, 2048] strip of -P
  is drained by a custom DVE op that in one pass writes the running
  column accumulator (out = max(strip, acc)) AND folds the strip's row
  maxes (accum_out = fold(Src0); hand-edited uop program). This touches
  each element exactly once (1 fresh el/lane/cycle) and computes BOTH
  reduction directions from one set of matmuls. The final column
  accumulator is DMA'd to the host (hides under batch 1's compute).

- Batch 1 (jobs 1-2, "two-direction paired"): -P and -P^T are computed
  separately; each strip is row-folded by a paired DVE read (PSUM lower
  half + ACT-copied upper half = 2 fresh els/lane/cycle). Same total
  DVE cycles as dual-fold, but needs NO column tail - the kernel ends
  at the last DVE instruction instead of a transpose+reduce chain.

The distance matrix comes from K-stacked bf16 matmuls (augmentation
trick, hi/lo split, K=15; PE time scales with output columns, not K)
with PE row-group rotation at partition offsets 0/32/64.
"""

import ml_dtypes
import numpy as np

import concourse.dve_ops as dve_ops
import concourse.mybir as mybir
import concourse.tile as tile
from concourse import bacc
from concourse.bass_utils import run_bass_kernel_spmd
from concourse.dve_ops import DveOp
from concourse.dve_spec import Spec, Src0, Src1, _has_src1, lower, maxx, minn
from concourse.dve_uop import DelayInp, DveOpSpec, OutPath, OutSel


def _fold_free(a):
    return np.max(a.astype(np.float32), axis=tuple(range(1, a.ndim))).reshape(
        a.shape[0], 1
    )


def _register_op(name, spec, edit=None):
    """Register a custom DVE op; optionally hand-edit the lowered uops.

    Edited programs are injected into dve_ops._COMPILE_CACHE so both the
    per-NEFF table generator and the instruction emitter use them.
    """
    for existing in dve_ops.OPS:
        if existing.name == name:
            return existing
    row = dve_ops._CUSTOM_DVE_ROW_BASE + len(dve_ops.OPS)
    shas = {}
    compiled = {}
    for ver in ("v3", "v4"):
        try:
            uops = lower(spec, ver=ver)
        except Exception:
            continue
        if edit is not None:
            uops = edit(uops, ver)
        s = DveOpSpec(name=name, opcode=row, uops=uops, rd1_en=_has_src1(spec))
        s.validate(ver)
        shas[ver] = s.sha(ver)
        compiled[ver] = s
    op = DveOp(name, spec, subdim=False, uops_sha=shas)
    dve_ops.OPS.append(op)
    dve_ops._SUB_OPCODE_FOR_NAME[op.name] = row
    dve_ops.CUSTOM_DVE_SPECS[op.name] = op.spec
    for ver, s in compiled.items():
        dve_ops._COMPILE_CACHE[(name, ver)] = s
    assert max(dve_ops._SUB_OPCODE_FOR_NAME.values()) < 0x20
    return op


def _register_max2():
    """out = max(Src0, Src1); accum_out = max-fold(out). (v1 paired fold)"""
    spec = Spec(
        body=maxx(Src0, Src1),
        accum=maxx,
        reference=lambda in0, in1, s0, s1, imm2: (
            np.maximum(in0.astype(np.float32), in1.astype(np.float32)),
            _fold_free(np.maximum(in0.astype(np.float32), in1.astype(np.float32))),
        ),
    )
    return _register_op("MAX2_REDUCE_ANT", spec)


def _register_maxpair_fold():
    """out = max(Src0, Src1); accum_out = max-fold(Src0) (dual-fold).

    Body min(Src0, max(Src0, Src1)) == Src0 makes lower() fold Src0;
    the hand edit reroutes `out` to the dp[0] pair max via delay lane 3.
    """

    def edit(uops, ver):
        assert len(uops) == 2, f"expected seed+steady, got {len(uops)}"
        seed, steady = uops
        assert steady.require_inp0 == 1, "uop order changed"
        for u in uops:
            for dp in u.datapath_config:
                dp.delay[3] = DelayInp.PREV_DELAY
                dp.delay_enable[3] = 1
        steady.datapath_config[1].delay[3] = DelayInp.PREV_ALU_OUT
        steady.out[OutPath.WR0_LO] = OutSel.DELAY_3
        return uops

    spec = Spec(
        body=minn(Src0, maxx(Src0, Src1)),
        accum=maxx,
        reference=lambda in0, in1, s0, s1, imm2: (
            np.maximum(in0.astype(np.float32), in1.astype(np.float32)),
            _fold_free(in0),
        ),
    )
    return _register_op("MAXPAIR_FOLD0_ANT", spec, edit)


def _register_copy_fold():
    """out = Src0 (accumulator init); accum_out = max-fold(Src0)."""
    spec = Spec(
        body=Src0,
        accum=maxx,
        reference=lambda in0, in1, s0, s1, imm2: (
            in0.astype(np.float32),
            _fold_free(in0),
        ),
    )
    return _register_op("COPY_FOLD0_ANT", spec)


MAX2_REDUCE = _register_max2()
MAXPAIR_FOLD = _register_maxpair_fold()
COPY_FOLD = _register_copy_fold()

B, N, M, D = 16, 2048, 2048, 3
N_CORES = 8
ROW_TILES = N // 128  # 16
COL_CHUNK = 512
KCAT = 15  # [hi; hi; lo] x [hi; lo; hi]
NJOBS = 3  # job0: batch0 single-P; jobs 1-2: batch1 dir0/dir1

_F32 = mybir.dt.float32
_BF16 = mybir.dt.bfloat16
_NP_BF16 = ml_dtypes.bfloat16

_cached_nc = None


def _build_nc():
    nc = bacc.Bacc("TRN2", target_bir_lowering=False, debug=False)
    # lhs+rhs packed per replica: one DMA per replica loads both
    ops = nc.dram_tensor("ops", [NJOBS, 3, KCAT, 2 * N], _BF16, kind="ExternalInput")
    # job0 lo/hi half row folds (host maxes the pair)
    rowm = nc.dram_tensor("rowm", [128, 2 * ROW_TILES], _F32, kind="ExternalOutput")
    # job0 final column accumulator, folded over rows on the host
    acc0 = nc.dram_tensor("acc0", [128, M], _F32, kind="ExternalOutput")
    # jobs 1-2 per-strip row folds
    maxs = nc.dram_tensor("maxs", [2, 128, ROW_TILES], _F32, kind="ExternalOutput")

    H = M // 2
    with tile.TileContext(nc) as tc:
        with (
            tc.tile_pool(name="inp", bufs=3) as inp_pool,
            tc.tile_pool(name="psum", bufs=2, space="PSUM") as psum_pool,
            tc.tile_pool(name="acc", bufs=1) as acc_pool,
            tc.tile_pool(name="res", bufs=3) as res_pool,
            tc.tile_pool(name="upper", bufs=8) as upper_pool,
        ):
            for j in range(NJOBS):
                ops_sb = inp_pool.tile([128, 2 * N], _BF16, tag="ops")
                engines = (nc.sync, nc.scalar, nc.gpsimd) if j == 0 else (nc.sync,) * 3
                for a, g in enumerate((0, 32, 64)):
                    engines[a].dma_start(ops_sb[g : g + KCAT, :], ops[j, a])

                if j == 0:
                    # --- single-P dual-fold: rows AND column accumulation ---
                    rowm_sb = res_pool.tile([128, 2 * ROW_TILES], _F32, tag="rowm")
                    acc_a = acc_pool.tile([128, M], _F32, tag="acc_a")
                    acc_b = acc_pool.tile([128, M], _F32, tag="acc_b")
                    for i in range(ROW_TILES):
                        lo_ps = psum_pool.tile([128, H], _F32, tag="lo")
                        hi_ps = psum_pool.tile([128, H], _F32, tag="hi")
                        li = slice(i * 128, (i + 1) * 128)
                        for c in range(4):
                            # strip 0 alternates two groups so its matmuls
                            # gate on two DMA queues, not three (ramp)
                            g = (c % 2) * 32 if i == 0 else ((i * 4 + c) % 3) * 32
                            cs = slice(c * COL_CHUNK, (c + 1) * COL_CHUNK)
                            dst = lo_ps if c < 2 else hi_ps
                            ds = slice((c % 2) * COL_CHUNK, (c % 2 + 1) * COL_CHUNK)
                            nc.tensor.matmul(
                                dst[:, ds],
                                ops_sb[g : g + KCAT, li],
                                ops_sb[g : g + KCAT, N + cs.start : N + cs.stop],
                                start=True,
                                stop=True,
                            )
                        cur, prv = (acc_a, acc_b) if i % 2 == 0 else (acc_b, acc_a)
                        for h, ps in ((0, lo_ps), (1, hi_ps)):
                            hs = slice(h * H, (h + 1) * H)
                            rs = slice(2 * i + h, 2 * i + h + 1)
                            if i == 0:
                                nc.vector._custom_dve(
                                    COPY_FOLD,
                                    out=cur[:, hs],
                                    in0=ps[:],
                                    accum_out=rowm_sb[:, rs],
                                )
                            else:
                                nc.vector._custom_dve(
                                    MAXPAIR_FOLD,
                                    out=cur[:, hs],
                                    in0=ps[:],
                                    in1=prv[:, hs],
                                    accum_out=rowm_sb[:, rs],
                                )
                    final_acc = acc_a if (ROW_TILES - 1) % 2 == 0 else acc_b
                    # column fold on host; this DMA hides under jobs 1-2
                    nc.sync.dma_start(acc0[:, 0:682], final_acc[:, 0:682])
                    nc.scalar.dma_start(acc0[:, 682:1364], final_acc[:, 682:1364])
                    nc.gpsimd.dma_start(acc0[:, 1364:M], final_acc[:, 1364:M])
                    nc.gpsimd.dma_start(rowm[:, :], rowm_sb[:])
                else:
                    # --- two-direction paired fold (v1 body): rows only ---
                    maxs_sb = res_pool.tile([128, ROW_TILES], _F32, tag="maxs")
                    for i in range(ROW_TILES):
                        dummy = upper_pool.tile([128, 1], _F32, tag="dummy")
                        hi_ps = psum_pool.tile([128, H], _F32, tag="hi")
                        lo_ps = psum_pool.tile([128, H], _F32, tag="lo")
                        li = slice(i * 128, (i + 1) * 128)
                        # upper-half chunks first so the ACT copy overlaps
                        # the PE filling the lower half
                        for k, (dst, half) in enumerate(
                            ((hi_ps, 0), (hi_ps, 1), (lo_ps, 0), (lo_ps, 1))
                        ):
                            g = ((i * 4 + k) % 3) * 32
                            c = 2 + k if k < 2 else k - 2
                            cs = slice(c * COL_CHUNK, (c + 1) * COL_CHUNK)
                            nc.tensor.matmul(
                                dst[:, half * COL_CHUNK : (half + 1) * COL_CHUNK],
                                ops_sb[g : g + KCAT, li],
                                ops_sb[g : g + KCAT, N + cs.start : N + cs.stop],
                                start=True,
                                stop=True,
                            )
                            if k == 1:
                                upper = upper_pool.tile([128, H], _F32, tag="upper")
                                nc.scalar.copy(upper[:], hi_ps[:])
                        nc.vector._custom_dve(
                            MAX2_REDUCE,
                            out=dummy.broadcast_to((128, H)),
                            in0=lo_ps[:],
                            in1=upper[:],
                            accum_out=maxs_sb[:, i : i + 1],
                        )
                    nc.sync.dma_start(maxs[j - 1], maxs_sb[:])
    nc.compile()
    return nc


def _get_nc():
    global _cached_nc
    if _cached_nc is None:
        _cached_nc = _build_nc()
    return _cached_nc


def _augment(a, b):
    """a: [n, 3], b: [m, 3] -> (lhsT [5, n], rhs [5, m]) float32, negated."""
    n = a.shape[0]
    m = b.shape[0]
    lhsT = np.empty((5, n), dtype=np.float32)
    lhsT[0:3] = -a.T
    lhsT[3] = -(a * a).sum(axis=1)
    lhsT[4] = -1.0
    rhs = np.empty((5, m), dtype=np.float32)
    rhs[0:3] = -2.0 * b.T
    rhs[3] = 1.0
    rhs[4] = (b * b).sum(axis=1)
    return lhsT, rhs


def _split_cat(lhs, rhs):
    """fp32 [J, 5, n] pairs -> packed K-stacked bf16 [J, 3, KCAT, 2n]."""
    lh = lhs.astype(_NP_BF16)
    ll = (lhs - lh.astype(np.float32)).astype(_NP_BF16)
    rh = rhs.astype(_NP_BF16)
    rl = (rhs - rh.astype(np.float32)).astype(_NP_BF16)
    lcat = np.concatenate([lh, lh, ll], axis=1)
    rcat = np.concatenate([rh, rl, rh], axis=1)
    packed = np.concatenate([lcat, rcat], axis=2)  # [J, KCAT, 2N]
    return np.ascontiguousarray(np.repeat(packed[:, None, :, :], 3, axis=1))


def _in_maps(predictions, targets):
    in_maps = []
    for core in range(N_CORES):
        b0, b1 = 2 * core, 2 * core + 1
        lhs = np.empty((NJOBS, 5, N), dtype=np.float32)
        rhs = np.empty((NJOBS, 5, M), dtype=np.float32)
        lhs[0], rhs[0] = _augment(predictions[b0], targets[b0])
        lhs[1], rhs[1] = _augment(predictions[b1], targets[b1])
        lhs[2], rhs[2] = _augment(targets[b1], predictions[b1])
        in_maps.append({"ops": _split_cat(lhs, rhs)})
    return in_maps


def _host_reduce(results):
    total = 0.0
    for core in range(N_CORES):
        r = results[core]
        pairs = r["rowm"].astype(np.float64).reshape(128, ROW_TILES, 2)
        total -= pairs.max(axis=-1).sum()
        total -= r["acc0"].astype(np.float64).max(axis=0).sum()
        total -= r["maxs"].astype(np.float64).sum()
    return np.float32(total)


def kernel(predictions, targets):
    predictions = np.asarray(predictions, dtype=np.float32)
    targets = np.asarray(targets, dtype=np.float32)

    nc = _get_nc()
    res = run_bass_kernel_spmd(
        nc, _in_maps(predictions, targets), core_ids=list(range(N_CORES))
    )
    return _host_reduce(res.results)


# revision 19
# speedup vs baseline: 1.1836x; 1.0171x over previous
"""Chamfer (AutoEncoder) loss on 8 Trainium2 NeuronCores.

Problem: predictions [16, 2048, 3], targets [16, 2048, 3] (float32).
loss = sum_b [ sum_i min_j ||x_bi - y_bj||^2 + sum_j min_i ||x_bi - y_bj||^2 ]

Strategy (v3: hybrid dual-fold / paired-fold)
---------------------------------------------
Data-parallel over the batch: 2 batches per core. The DVE is the only
engine that can compute max (HW probes: ACT accumulates sums only,
GPSIMD rejects tensor-tensor ops, no DVE 2x perf modes engage), so
every distance-matrix element must stream through it; the whole design
minimizes DVE instruction count and kernel tail.

- Batch 0 (job 0, "single-P dual-fold"): each [128, 2048] strip of -P
  is drained by a custom DVE op that in one pass writes the running
  column accumulator (out = max(strip, acc)) AND folds the strip's row
  maxes (accum_out = fold(Src0); hand-edited uop program). This touches
  each element exactly once (1 fresh el/lane/cycle) and computes BOTH
  reduction directions from one set of matmuls. The final column
  accumulator is DMA'd to the host (hides under batch 1's compute).

- Batch 1 (jobs 1-2, "two-direction paired"): -P and -P^T are computed
  separately; each strip is row-folded by a paired DVE read (PSUM lower
  half + ACT-copied upper half = 2 fresh els/lane/cycle). Same total
  DVE cycles as dual-fold, but needs NO column tail - the kernel ends
  at the last DVE instruction instead of a transpose+reduce chain.

The distance matrix comes from K-stacked bf16 matmuls (augmentation
trick, hi/lo split, K=15; PE time scales with output columns, not K)
with PE row-group rotation at partition offsets 0/32/64.
"""

import ml_dtypes
import numpy as np

import concourse.dve_ops as dve_ops
import concourse.mybir as mybir
import concourse.tile as tile
from concourse import bacc
from concourse.bass_utils import run_bass_kernel_spmd
from concourse.dve_ops import DveOp
from concourse.dve_spec import Spec, Src0, Src1, _has_src1, lower, maxx, minn
from concourse.dve_uop import DelayInp, DveOpSpec, OutPath, OutSel


def _fold_free(a):
    return np.max(a.astype(np.float32), axis=tuple(range(1, a.ndim))).reshape(
        a.shape[0], 1
    )


def _register_op(name, spec, edit=None):
    """Register a custom DVE op; optionally hand-edit the lowered uops.

    Edited programs are injected into dve_ops._COMPILE_CACHE so both the
    per-NEFF table generator and the instruction emitter use them.
    """
    for existing in dve_ops.OPS:
        if existing.name == name:
            return existing
    row = dve_ops._CUSTOM_DVE_ROW_BASE + len(dve_ops.OPS)
    shas = {}
    compiled = {}
    for ver in ("v3", "v4"):
        try:
            uops = lower(spec, ver=ver)
        except Exception:
            continue
        if edit is not None:
            uops = edit(uops, ver)
        s = DveOpSpec(name=name, opcode=row, uops=uops, rd1_en=_has_src1(spec))
        s.validate(ver)
        shas[ver] = s.sha(ver)
        compiled[ver] = s
    op = DveOp(name, spec, subdim=False, uops_sha=shas)
    dve_ops.OPS.append(op)
    dve_ops._SUB_OPCODE_FOR_NAME[op.name] = row
    dve_ops.CUSTOM_DVE_SPECS[op.name] = op.spec
    for ver, s in compiled.items():
        dve_ops._COMPILE_CACHE[(name, ver)] = s
    assert max(dve_ops._SUB_OPCODE_FOR_NAME.values()) < 0x20
    return op


def _register_max2():
    """out = max(Src0, Src1); accum_out = max-fold(out). (v1 paired fold)"""
    spec = Spec(
        body=maxx(Src0, Src1),
        accum=maxx,
        reference=lambda in0, in1, s0, s1, imm2: (
            np.maximum(in0.astype(np.float32), in1.astype(np.float32)),
            _fold_free(np.maximum(in0.astype(np.float32), in1.astype(np.float32))),
        ),
    )
    return _register_op("MAX2_REDUCE_ANT", spec)


def _register_maxpair_fold():
    """out = max(Src0, Src1); accum_out = max-fold(Src0) (dual-fold).

    Body min(Src0, max(Src0, Src1)) == Src0 makes lower() fold Src0;
    the hand edit reroutes `out` to the dp[0] pair max via delay lane 3.
    """

    def edit(uops, ver):
        assert len(uops) == 2, f"expected seed+steady, got {len(uops)}"
        seed, steady = uops
        assert steady.require_inp0 == 1, "uop order changed"
        for u in uops:
            for dp in u.datapath_config:
                dp.delay[3] = DelayInp.PREV_DELAY
                dp.delay_enable[3] = 1
        steady.datapath_config[1].delay[3] = DelayInp.PREV_ALU_OUT
        steady.out[OutPath.WR0_LO] = OutSel.DELAY_3
        return uops

    spec = Spec(
        body=minn(Src0, maxx(Src0, Src1)),
        accum=maxx,
        reference=lambda in0, in1, s0, s1, imm2: (
            np.maximum(in0.astype(np.float32), in1.astype(np.float32)),
            _fold_free(in0),
        ),
    )
    return _register_op("MAXPAIR_FOLD0_ANT", spec, edit)


def _register_copy_fold():
    """out = Src0 (accumulator init); accum_out = max-fold(Src0)."""
    spec = Spec(
        body=Src0,
        accum=maxx,
        reference=lambda in0, in1, s0, s1, imm2: (
            in0.astype(np.float32),
            _fold_free(in0),
        ),
    )
    return _register_op("COPY_FOLD0_ANT", spec)


MAX2_REDUCE = _register_max2()
MAXPAIR_FOLD = _register_maxpair_fold()
COPY_FOLD = _register_copy_fold()

B, N, M, D = 16, 2048, 2048, 3
N_CORES = 8
ROW_TILES = N // 128  # 16
COL_CHUNK = 512
KCAT = 15  # [hi; hi; lo] x [hi; lo; hi]
NJOBS = 3  # job0: batch0 single-P; jobs 1-2: batch1 dir0/dir1

_F32 = mybir.dt.float32
_BF16 = mybir.dt.bfloat16
_NP_BF16 = ml_dtypes.bfloat16

_cached_nc = None


def _build_nc():
    nc = bacc.Bacc("TRN2", target_bir_lowering=False, debug=False)
    # lhs+rhs packed per replica: one DMA per replica loads both
    ops = nc.dram_tensor("ops", [NJOBS, 3, KCAT, 2 * N], _BF16, kind="ExternalInput")
    # job0 lo/hi half row folds (host maxes the pair)
    rowm = nc.dram_tensor("rowm", [128, 2 * ROW_TILES], _F32, kind="ExternalOutput")
    # job0 final column accumulator, folded over rows on the host
    acc0 = nc.dram_tensor("acc0", [128, M], _F32, kind="ExternalOutput")
    # jobs 1-2 per-strip row folds
    maxs = nc.dram_tensor("maxs", [2, 128, ROW_TILES], _F32, kind="ExternalOutput")

    H = M // 2
    with tile.TileContext(nc) as tc:
        with (
            tc.tile_pool(name="inp", bufs=3) as inp_pool,
            tc.tile_pool(name="psum", bufs=2, space="PSUM") as psum_pool,
            tc.tile_pool(name="acc", bufs=1) as acc_pool,
            tc.tile_pool(name="res", bufs=3) as res_pool,
            tc.tile_pool(name="upper", bufs=8) as upper_pool,
        ):
            for j in range(NJOBS):
                ops_sb = inp_pool.tile([128, 2 * N], _BF16, tag="ops")
                engines = (nc.sync, nc.scalar, nc.gpsimd) if j == 0 else (nc.sync,) * 3
                if j == 0:
                    # Priority slices: strip 0 (groups 0/32) needs only lhs
                    # cols 0:128 and its two rhs chunks per replica - load
                    # those first so the first matmuls start ~3us earlier.
                    for a, g in ((0, 0), (1, 32)):
                        eng = engines[a]
                        eng.dma_start(
                            ops_sb[g : g + KCAT, 0:128], ops[j, a][:, 0:128]
                        )
                        c0 = N + a * COL_CHUNK
                        eng.dma_start(
                            ops_sb[g : g + KCAT, c0 : c0 + COL_CHUNK],
                            ops[j, a][:, c0 : c0 + COL_CHUNK],
                        )
                        c2 = c0 + 2 * COL_CHUNK
                        eng.dma_start(
                            ops_sb[g : g + KCAT, c2 : c2 + COL_CHUNK],
                            ops[j, a][:, c2 : c2 + COL_CHUNK],
                        )
                        eng.dma_start(
                            ops_sb[g : g + KCAT, 128:N], ops[j, a][:, 128:N]
                        )
                        o = N + (1 - a) * COL_CHUNK
                        eng.dma_start(
                            ops_sb[g : g + KCAT, o : o + COL_CHUNK],
                            ops[j, a][:, o : o + COL_CHUNK],
                        )
                        o2 = o + 2 * COL_CHUNK
                        eng.dma_start(
                            ops_sb[g : g + KCAT, o2 : o2 + COL_CHUNK],
                            ops[j, a][:, o2 : o2 + COL_CHUNK],
                        )
                    engines[2].dma_start(ops_sb[64 : 64 + KCAT, :], ops[j, 2])
                else:
                    for a, g in enumerate((0, 32, 64)):
                        engines[a].dma_start(ops_sb[g : g + KCAT, :], ops[j, a])

                if j == 0:
                    # --- single-P dual-fold: rows AND column accumulation ---
                    rowm_sb = res_pool.tile([128, 2 * ROW_TILES], _F32, tag="rowm")
                    acc_a = acc_pool.tile([128, M], _F32, tag="acc_a")
                    acc_b = acc_pool.tile([128, M], _F32, tag="acc_b")
                    for i in range(ROW_TILES):
                        lo_ps = psum_pool.tile([128, H], _F32, tag="lo")
                        hi_ps = psum_pool.tile([128, H], _F32, tag="hi")
                        li = slice(i * 128, (i + 1) * 128)
                        for c in range(4):
                            # strip 0 alternates two groups so its matmuls
                            # gate on two DMA queues, not three (ramp)
                            g = (c % 2) * 32 if i == 0 else ((i * 4 + c) % 3) * 32
                            cs = slice(c * COL_CHUNK, (c + 1) * COL_CHUNK)
                            dst = lo_ps if c < 2 else hi_ps
                            ds = slice((c % 2) * COL_CHUNK, (c % 2 + 1) * COL_CHUNK)
                            nc.tensor.matmul(
                                dst[:, ds],
                                ops_sb[g : g + KCAT, li],
                                ops_sb[g : g + KCAT, N + cs.start : N + cs.stop],
                                start=True,
                                stop=True,
                            )
                        cur, prv = (acc_a, acc_b) if i % 2 == 0 else (acc_b, acc_a)
                        for h, ps in ((0, lo_ps), (1, hi_ps)):
                            hs = slice(h * H, (h + 1) * H)
                            rs = slice(2 * i + h, 2 * i + h + 1)
                            if i == 0:
                                nc.vector._custom_dve(
                                    COPY_FOLD,
                                    out=cur[:, hs],
                                    in0=ps[:],
                                    accum_out=rowm_sb[:, rs],
                                )
                            else:
                                nc.vector._custom_dve(
                                    MAXPAIR_FOLD,
                                    out=cur[:, hs],
                                    in0=ps[:],
                                    in1=prv[:, hs],
                                    accum_out=rowm_sb[:, rs],
                                )
                    final_acc = acc_a if (ROW_TILES - 1) % 2 == 0 else acc_b
                    # column fold on host; this DMA hides under jobs 1-2
                    nc.sync.dma_start(acc0[:, 0:682], final_acc[:, 0:682])
                    nc.scalar.dma_start(acc0[:, 682:1364], final_acc[:, 682:1364])
                    nc.gpsimd.dma_start(acc0[:, 1364:M], final_acc[:, 1364:M])
                    nc.gpsimd.dma_start(rowm[:, :], rowm_sb[:])
                else:
                    # --- two-direction paired fold (v1 body): rows only ---
                    maxs_sb = res_pool.tile([128, ROW_TILES], _F32, tag="maxs")
                    for i in range(ROW_TILES):
                        dummy = upper_pool.tile([128, 1], _F32, tag="dummy")
                        # strip 0: take the other ring's slot (freed one DVE
                        # instruction earlier) so the ACT copy chain starts
                        # sooner at the job boundary
                        t_hi, t_lo = ("lo", "hi") if i == 0 else ("hi", "lo")
                        hi_ps = psum_pool.tile([128, H], _F32, tag=t_hi)
                        lo_ps = psum_pool.tile([128, H], _F32, tag=t_lo)
                        li = slice(i * 128, (i + 1) * 128)
                        # upper-half chunks first so the ACT copy overlaps
                        # the PE filling the lower half
                        for k, (dst, half) in enumerate(
                            ((hi_ps, 0), (hi_ps, 1), (lo_ps, 0), (lo_ps, 1))
                        ):
                            g = ((i * 4 + k) % 3) * 32
                            c = 2 + k if k < 2 else k - 2
                            cs = slice(c * COL_CHUNK, (c + 1) * COL_CHUNK)
                            nc.tensor.matmul(
                                dst[:, half * COL_CHUNK : (half + 1) * COL_CHUNK],
                                ops_sb[g : g + KCAT, li],
                                ops_sb[g : g + KCAT, N + cs.start : N + cs.stop],
                                start=True,
                                stop=True,
                            )
                            if k == 1:
                                upper = upper_pool.tile([128, H], _F32, tag="upper")
                                nc.scalar.copy(upper[:], hi_ps[:])
                        nc.vector._custom_dve(
                            MAX2_REDUCE,
                            out=dummy.broadcast_to((128, H)),
                            in0=lo_ps[:],
                            in1=upper[:],
                            accum_out=maxs_sb[:, i : i + 1],
                        )
                    nc.sync.dma_start(maxs[j - 1], maxs_sb[:])
    nc.compile()
    return nc


def _get_nc():
    global _cached_nc
    if _cached_nc is None:
        _cached_nc = _build_nc()
    return _cached_nc


def _augment(a, b):
    """a: [n, 3], b: [m, 3] -> (lhsT [5, n], rhs [5, m]) float32, negated."""
    n = a.shape[0]
    m = b.shape[0]
    lhsT = np.empty((5, n), dtype=np.float32)
    lhsT[0:3] = -a.T
    lhsT[3] = -(a * a).sum(axis=1)
    lhsT[4] = -1.0
    rhs = np.empty((5, m), dtype=np.float32)
    rhs[0:3] = -2.0 * b.T
    rhs[3] = 1.0
    rhs[4] = (b * b).sum(axis=1)
    return lhsT, rhs


def _split_cat(lhs, rhs):
    """fp32 [J, 5, n] pairs -> packed K-stacked bf16 [J, 3, KCAT, 2n]."""
    lh = lhs.astype(_NP_BF16)
    ll = (lhs - lh.astype(np.float32)).astype(_NP_BF16)
    rh = rhs.astype(_NP_BF16)
    rl = (rhs - rh.astype(np.float32)).astype(_NP_BF16)
    lcat = np.concatenate([lh, lh, ll], axis=1)
    rcat = np.concatenate([rh, rl, rh], axis=1)
    packed = np.concatenate([lcat, rcat], axis=2)  # [J, KCAT, 2N]
    return np.ascontiguousarray(np.repeat(packed[:, None, :, :], 3, axis=1))


def _in_maps(predictions, targets):
    in_maps = []
    for core in range(N_CORES):
        b0, b1 = 2 * core, 2 * core + 1
        lhs = np.empty((NJOBS, 5, N), dtype=np.float32)
        rhs = np.empty((NJOBS, 5, M), dtype=np.float32)
        lhs[0], rhs[0] = _augment(predictions[b0], targets[b0])
        lhs[1], rhs[1] = _augment(predictions[b1], targets[b1])
        lhs[2], rhs[2] = _augment(targets[b1], predictions[b1])
        in_maps.append({"ops": _split_cat(lhs, rhs)})
    return in_maps


def _host_reduce(results):
    total = 0.0
    for core in range(N_CORES):
        r = results[core]
        pairs = r["rowm"].astype(np.float64).reshape(128, ROW_TILES, 2)
        total -= pairs.max(axis=-1).sum()
        total -= r["acc0"].astype(np.float64).max(axis=0).sum()
        total -= r["maxs"].astype(np.float64).sum()
    return np.float32(total)


def kernel(predictions, targets):
    predictions = np.asarray(predictions, dtype=np.float32)
    targets = np.asarray(targets, dtype=np.float32)

    nc = _get_nc()
    res = run_bass_kernel_spmd(
        nc, _in_maps(predictions, targets), core_ids=list(range(N_CORES))
    )
    return _host_reduce(res.results)
